# revision 21
# baseline (speedup 1.0000x reference)
"""Trainium2 Bass kernel for nn_Attention (pooling attention head).

Reference computation (per batch b):
    score[t]  = hidden[t,:] @ W_score @ hidden[-1,:]        # via u = W_score @ h_t
    attn      = softmax(score)
    context   = sum_t attn[t] * hidden[t,:]
    out       = tanh(concat(context, h_t) @ W_out)

Key optimization: the reference computes (hidden @ W_score) [B,T,H] first
(69 GFLOP); we reassociate to u = W_score @ h_t (34 MFLOP) and then
score = hidden @ u, so the kernel is a single memory-bound streaming pass
over hidden_states.

Sharding: data-parallel over batch, 8 batches per NeuronCore, no
collectives. Each core returns its [8, 128] slice of the output.

Layout: partition p holds t-rows p*16 .. p*16+15 (32KB contiguous HBM
reads per partition); column j of S/P maps to t = p*16 + j. The softmax
is order-agnostic and the PE contraction sums over all (p, j), so the
remapping is transparent.

Engine schedule (steady state ~10us/batch, DVE/ACT balanced):
  SWDGE ring: W_score first (avoids starvation), then 2x 2MB cast-DMA
              (fp32->bf16 inline) per batch
  DVE:        8x fused mul+reduce (scalar_tensor_tensor, 1x) +
              8x 2x-mode mul + softmax stats
  ACT:        8x copy-accum score reductions + exp + context copy
  PE:         16x (1-col ldweights + N=512 matmul) context + stat
              transposes + scaled one-hot scatter of the context row
The batch loop is software-pipelined one deep: batch b's score ops are
emitted before batch b-1's softmax/context so no engine FIFO head-of-line
blocks the next batch's work.
"""

import os

os.environ.setdefault("MYCRO_LOCAL_CACHE", "1")

from contextlib import ExitStack

import numpy as np

import concourse.bass as bass
import concourse.tile as tile
from concourse import bacc, mybir
from concourse.bass_utils import run_bass_kernel_spmd
from concourse.masks import make_identity
from concourse.tile_rust import add_dep_helper

B, T, H, UNITS = 64, 2048, 512, 128
NCORES = 8
BL = B // NCORES  # local batches per core
NT = T // 128  # 16 t-tiles per batch

F32 = mybir.dt.float32
BF16 = mybir.dt.bfloat16

DMA_AHEAD = 3  # batches of hid prefetch issued ahead of the score loop


def _kernel_body(tc: tile.TileContext, out, hs, ws, wo):
    nc = tc.nc
    with ExitStack() as ctx:
        singles = ctx.enter_context(tc.tile_pool(name="singles", bufs=1))
        hid_pool = ctx.enter_context(tc.tile_pool(name="hid", bufs=10))
        work = ctx.enter_context(tc.tile_pool(name="work", bufs=3))
        small = ctx.enter_context(tc.tile_pool(name="small", bufs=2))
        ps_setup = ctx.enter_context(
            tc.tile_pool(name="ps_setup", bufs=2, space="PSUM")
        )
        ps_stat = ctx.enter_context(tc.tile_pool(name="ps_stat", bufs=2, space="PSUM"))
        ps_ctx = ctx.enter_context(tc.tile_pool(name="ps_ctx", bufs=2, space="PSUM"))
        ps_keep = ctx.enter_context(tc.tile_pool(name="ps_keep", bufs=1, space="PSUM"))
        dram = ctx.enter_context(tc.tile_pool(name="dram", bufs=1, space="DRAM"))

        ident = singles.tile([128, 128], F32)
        make_identity(nc, ident)

        # ---- input loads ------------------------------------------------
        # W_score goes FIRST on the gpsimd (SWDGE) ring so it completes
        # before the hid stream hogs the SDMA engines; h_t (tiny) on sync;
        # W_out (descriptor-heavy, needed only at the end) on the scalar
        # HWDGE ring.
        ws_sb = singles.tile([128, 4, H], F32)  # W_score rows r*128+p
        nc.gpsimd.dma_start(out=ws_sb, in_=ws.rearrange("(r p) k -> p r k", p=128))
        ht_sb = singles.tile([BL, H], F32)  # h_t = hidden[:, -1, :]
        nc.sync.dma_start(out=ht_sb, in_=hs[:, T - 1, :])
        wout_sb = singles.tile([128, 8, UNITS], F32)  # W_out rows c*128+p
        nc.scalar.dma_start(out=wout_sb, in_=wo.rearrange("(c p) j -> p c j", p=128))

        # ---- h_t^T: htT_sb[p, c, b] = h_t[b, c*128+p]
        htT_sb = singles.tile([128, 4, BL], F32)
        for c in range(4):
            pst = ps_setup.tile([128, BL], F32, tag="setup")
            nc.tensor.transpose(
                pst, ht_sb[:, c * 128 : (c + 1) * 128], ident[:BL, :BL]
            )
            nc.vector.tensor_copy(out=htT_sb[:, c, :], in_=pst)

        # ---- W_score^T (PE transposes): wsT_sb[p, kc, h] = W_score[h, kc*128+p]
        # Emitted kc-major so each wsT k-chunk completes as a unit and the
        # matching u matmul can issue right behind it.
        wsT_sb = singles.tile([128, 4, H], F32)
        psu = ps_setup.tile([BL, H], F32, tag="psu", bufs=1)
        for c in range(4):
            for r in range(4):
                pst = ps_setup.tile([128, 128], F32, tag="setup")
                nc.tensor.transpose(pst, ws_sb[:, r, c * 128 : (c + 1) * 128], ident)
                nc.vector.tensor_copy(
                    out=wsT_sb[:, c, r * 128 : (r + 1) * 128], in_=pst
                )
            # u rows for all batches: u[b, m] += sum_{k in chunk c} h_t[b,k] W^T[k,m]
            nc.tensor.matmul(
                psu,
                lhsT=htT_sb[:, c, :],
                rhs=wsT_sb[:, c, :],
                start=(c == 0),
                stop=(c == 3),
            )
        u_rows_bf = singles.tile([BL, H], BF16)
        nc.vector.tensor_copy(out=u_rows_bf, in_=psu)

        # stage u rows to DRAM, then DMA-broadcast each row to all 128
        # partitions (DRE replication) on the sync ring
        u_dram = dram.tile([BL, H], BF16)
        nc.sync.dma_start(out=u_dram, in_=u_rows_bf)
        u_bc_all = singles.tile([128, BL, H], BF16)
        bc_last = None
        for b in range(BL):
            bc_last = nc.sync.dma_start(
                out=u_bc_all[:, b, :],
                in_=u_dram[b : b + 1, :].to_broadcast([128, H]),
            )

        # preT_sb[p, c, b]: transposed concat(context, h_t); ht half now
        preT_sb = singles.tile([128, 8, BL], F32)
        for c in range(4):
            nc.vector.tensor_copy(out=preT_sb[:, 4 + c, :], in_=htT_sb[:, c, :])

        # ones row for PE-based partition broadcasts
        ones_sb = singles.tile([1, 128], F32)
        nc.vector.memset(ones_sb, 1.0)

        # accumulated, normalized context rows (batch b in partition b);
        # single accumulation group — PSUM allows only one pending group
        # per zero region (bank)
        ctx8_ps = ps_keep.tile([BL, H], F32)

        # ---- software-pipelined batch loop ------------------------------
        hid_tiles = {}

        def issue_dma(b):
            hs_v = hs[b].rearrange("(p n) h -> p n h", p=128)
            halves = []
            for half in range(2):
                hid_bf = hid_pool.tile([128, NT // 2, H], BF16, tag="hid")
                di = nc.gpsimd.dma_start(
                    out=hid_bf, in_=hs_v[:, half * 8 : half * 8 + 8, :]
                )
                if b == 0 and half == 0:
                    # hold the hid stream until the u broadcasts have landed
                    # so the tiny setup DMAs aren't starved by the 2MB casts
                    add_dep_helper(
                        di.ins,
                        bc_last.ins,
                        sync=True,
                        reason="u broadcast before hid stream",
                    )
                halves.append(hid_bf)
            hid_tiles[b] = halves

        def issue_score(b):
            halves = hid_tiles[b]
            S = small.tile([128, NT], F32, tag="S")
            dump_v = work.tile([128, H], BF16, tag="dump_v")
            dump_a = work.tile([128, H], BF16, tag="dump_a")
            for j in range(NT):
                src = halves[j // 8][:, j % 8, :]
                if j % 2 == 0:
                    nc.vector.scalar_tensor_tensor(
                        out=dump_v,
                        in0=src,
                        scalar=1.0,
                        in1=u_bc_all[:, b, :],
                        op0=mybir.AluOpType.mult,
                        op1=mybir.AluOpType.mult,
                        accum_out=S[:, j : j + 1],
                    )
                else:
                    prod = work.tile([128, H], BF16, tag="prod")
                    nc.vector.tensor_mul(prod, src, u_bc_all[:, b, :])
                    nc.scalar.activation(
                        dump_a,
                        prod,
                        mybir.ActivationFunctionType.Copy,
                        accum_out=S[:, j : j + 1],
                    )
            return S

        def issue_finish(b, S):
            halves = hid_tiles[b]
            # softmax stats: global max via PE transpose + ones broadcast
            m_row = small.tile([128, 1], F32, tag="m_row")
            nc.vector.reduce_max(m_row, S, axis=mybir.AxisListType.X)
            mT_ps = ps_stat.tile([1, 128], F32, tag="stat")
            nc.tensor.transpose(mT_ps, m_row, ident)
            M_sb = small.tile([1, 1], F32, tag="M_sb")
            nc.vector.reduce_max(M_sb, mT_ps[0:1, :], axis=mybir.AxisListType.X)
            Mb_ps = ps_stat.tile([128, 1], F32, tag="stat")
            nc.tensor.matmul(Mb_ps, lhsT=ones_sb, rhs=M_sb, start=True, stop=True)
            nm = small.tile([128, 1], F32, tag="nm")
            nc.vector.tensor_scalar_mul(nm, Mb_ps, -1.0)

            P = small.tile([128, NT], BF16, tag="P")
            l_row = small.tile([128, 1], F32, tag="l_row")
            nc.scalar.activation(
                P,
                S,
                mybir.ActivationFunctionType.Exp,
                bias=nm,
                scale=1.0,
                accum_out=l_row,
            )
            lT_ps = ps_stat.tile([1, 128], F32, tag="stat")
            nc.tensor.transpose(lT_ps, l_row, ident)
            L_sb = small.tile([1, 1], F32, tag="L_sb")
            nc.vector.reduce_sum(L_sb, lT_ps[0:1, :], axis=mybir.AxisListType.X)

            # context row (unnormalized): ctx[0,h] = sum_j sum_p P[p,j]*hid_j[p,h]
            ctx_ps = ps_ctx.tile([1, H], F32, tag="ctx")
            for j in range(NT):
                nc.tensor.matmul(
                    ctx_ps,
                    lhsT=P[:, j : j + 1],
                    rhs=halves[j // 8][:, j % 8, :],
                    start=(j == 0),
                    stop=(j == NT - 1),
                )
            ctx_row = work.tile([1, H], F32, tag="ctx_row")
            nc.scalar.copy(ctx_row, ctx_ps)

            # scaled one-hot scatter: ctx8_ps[b, :] += (1/L) * ctx_row
            e_b = small.tile([1, BL], F32, tag="e_b")
            nc.vector.memset(e_b, 0.0)
            nc.vector.reciprocal(e_b[0:1, b : b + 1], L_sb)
            nc.tensor.matmul(
                ctx8_ps,
                lhsT=e_b,
                rhs=ctx_row,
                start=(b == 0),
                stop=(b == BL - 1),
            )

        for b in range(min(DMA_AHEAD, BL)):
            issue_dma(b)
        S_prev = issue_score(0)
        for b in range(1, BL):
            if b - 1 + DMA_AHEAD < BL:
                issue_dma(b - 1 + DMA_AHEAD)
            S_cur = issue_score(b)
            issue_finish(b - 1, S_prev)
            S_prev = S_cur
        issue_finish(BL - 1, S_prev)

        # ---- final: out = tanh(concat(ctx, h_t) @ W_out) ----------------
        ctx_sb = singles.tile([BL, H], F32)
        nc.scalar.copy(ctx_sb, ctx8_ps)
        for c in range(4):
            pst = ps_setup.tile([128, BL], F32, tag="setup")
            nc.tensor.transpose(
                pst, ctx_sb[:, c * 128 : (c + 1) * 128], ident[:BL, :BL]
            )
            nc.vector.tensor_copy(out=preT_sb[:, c, :], in_=pst)

        psum_out = ps_setup.tile([BL, UNITS], F32, tag="setup")
        for c in range(8):
            nc.tensor.matmul(
                psum_out,
                lhsT=preT_sb[:, c, :],
                rhs=wout_sb[:, c, :],
                start=(c == 0),
                stop=(c == 7),
            )
        y_sb = small.tile([BL, UNITS], F32, tag="y")
        nc.scalar.activation(y_sb, psum_out, mybir.ActivationFunctionType.Tanh)
        nc.sync.dma_start(out=out, in_=y_sb)


def build_nc():
    nc = bacc.Bacc(
        "TRN2",
        target_bir_lowering=False,
        debug=False,
        enable_asserts=False,
        num_devices=NCORES,
    )
    hs = nc.dram_tensor(
        "hidden_states", [BL, T, H], F32, kind="ExternalInput"
    ).ap()
    ws = nc.dram_tensor("W_score", [H, H], F32, kind="ExternalInput").ap()
    wo = nc.dram_tensor("W_out", [2 * H, UNITS], F32, kind="ExternalInput").ap()
    out = nc.dram_tensor("out", [BL, UNITS], F32, kind="ExternalOutput").ap()

    with tile.TileContext(nc) as tc:
        _kernel_body(tc, out, hs, ws, wo)
    nc.compile()
    return nc


_NC = None


def _get_nc():
    global _NC
    if _NC is None:
        _NC = build_nc()
    return _NC


def make_in_maps(hidden_states, W_score, W_out):
    hidden_states = np.ascontiguousarray(
        np.asarray(hidden_states, dtype=np.float32)
    )
    W_score = np.ascontiguousarray(np.asarray(W_score, dtype=np.float32))
    W_out = np.ascontiguousarray(np.asarray(W_out, dtype=np.float32))
    return [
        {
            "hidden_states": hidden_states[i * BL : (i + 1) * BL],
            "W_score": W_score,
            "W_out": W_out,
        }
        for i in range(NCORES)
    ]


def kernel(hidden_states, W_score, W_out):
    nc = _get_nc()
    in_maps = make_in_maps(hidden_states, W_score, W_out)
    res = run_bass_kernel_spmd(nc, in_maps, core_ids=list(range(NCORES)))
    return np.concatenate([res.results[i]["out"] for i in range(NCORES)], axis=0)


if __name__ == "__main__":
    build_nc()
    print("compile OK")
